# Initial kernel scaffold
#
"""Multi-head self-attention (SAGAN-style, 1x1-conv projections) on 8 Trainium2 cores.

Problem: x [2, 256, 64, 64], 8 heads, cph=32, L=4096 tokens per batch element.
  q/k/v = 1x1 conv projections of x; att = softmax_j(k_i . q_j); out_i = sum_j att_ij v_j;
  y = gamma * (Wl @ out + bl) + x

Sharding: output-token split — core c owns (n = c//4, token chunk c%4 of 1024).
Each core needs: full Q and V for its n, K only for its chunk. No collectives.

Per-core kernel layout (all matmuls bf16 with fp32 PSUM accumulation):
  S^T[j, i] = sum_c q'[c,j] k[c,i]   (q' pre-scaled by cph^-0.5; softmax over j)
    -> 4 heads row-packed in the 128x128 PE array (K=32 each, tile_position=(32h,0))
  P^T = exp(S^T)  on ACT, PSUM -> SBUF bf16 (scores bounded ~0.8, no max-subtraction
    needed). This is the bottleneck: 33.5M exps/core at 1 elem/cycle/lane.
  AV[d, i] = sum_j v^T[j,d] P^T[j,i]  -> 4 heads col-packed (M=32, tile_position=(0,32h))
  denom[i] = sum_j P^T[j,i]           -> ones-vector matmuls, M=1, col-packed
  attn = AV * (1/denom broadcast via stream_shuffle)
  y = Wl_gamma @ attn + bl_gamma + x_residual   (gamma folded into Wl/bl host-side)
"""
import numpy as np
import ml_dtypes

bf16 = ml_dtypes.bfloat16

N, C, H, W = 2, 256, 64, 64
L = H * W            # 4096
NH = 8               # heads
CPH = C // NH        # 32
NCORES = 8
CHUNK = 1024         # output tokens per core (L / 4)
P = 128

_cache = {}


def _build():
    import concourse.mybir as mybir
    import concourse.tile as tile
    from concourse import bacc

    FP32 = mybir.dt.float32
    BF16 = mybir.dt.bfloat16
    EXP = mybir.ActivationFunctionType.Exp
    ADD = mybir.AluOpType.add

    nc = bacc.Bacc("TRN2", target_bir_lowering=False, debug=False,
                   num_devices=NCORES)

    # ---- DRAM I/O (per-core shapes; data differs per core, program is SPMD)
    xb_d = nc.dram_tensor("xb", [C, L], BF16, kind="ExternalInput").ap()
    xk_d = nc.dram_tensor("xk", [C, CHUNK], BF16, kind="ExternalInput").ap()
    xres_d = nc.dram_tensor("xres", [C, CHUNK], FP32, kind="ExternalInput").ap()
    wq_d = nc.dram_tensor("wqt", [C, C], BF16, kind="ExternalInput").ap()
    wk_d = nc.dram_tensor("wkt", [C, C], BF16, kind="ExternalInput").ap()
    wv_d = nc.dram_tensor("wvt", [C, C], BF16, kind="ExternalInput").ap()
    wl_d = nc.dram_tensor("wlt", [C, C], BF16, kind="ExternalInput").ap()
    bq_d = nc.dram_tensor("bq2", [C, 1], FP32, kind="ExternalInput").ap()
    bk_d = nc.dram_tensor("bk2", [C, 1], FP32, kind="ExternalInput").ap()
    bl_d = nc.dram_tensor("bl2", [C, 1], FP32, kind="ExternalInput").ap()
    bv_d = nc.dram_tensor("bv2", [1, C], BF16, kind="ExternalInput").ap()
    y_d = nc.dram_tensor("y", [C, CHUNK], FP32, kind="ExternalOutput").ap()

    JT = L // P          # 32 j-tiles
    TS = L // 512        # 8 token slices for Q projection
    IH = CHUNK // 512    # 2 i-halves

    with tile.TileContext(nc) as tc:
        with tc.tile_pool(name="consts", bufs=1) as consts, \
             tc.tile_pool(name="data", bufs=1) as data, \
             tc.tile_pool(name="ptp", bufs=4) as ptp, \
             tc.tile_pool(name="ep", bufs=2) as ep, \
             tc.tile_pool(name="attnp", bufs=2) as attnp:

            # ---- constants / weights
            wq_sb, wk_sb, wv_sb, wl_sb = [], [], [], []
            bq_sb, bk_sb, bl_sb = [], [], []
            for ct in range(2):
                for lst, dram, nm in ((wq_sb, wq_d, "wq"), (wk_sb, wk_d, "wk"),
                                      (wv_sb, wv_d, "wv"), (wl_sb, wl_d, "wl")):
                    t = consts.tile([P, C], BF16, name=f"{nm}{ct}")
                    nc.sync.dma_start(out=t, in_=dram[ct * P:(ct + 1) * P, :])
                    lst.append(t)
                for lst, dram, nm in ((bq_sb, bq_d, "bq"), (bk_sb, bk_d, "bk"),
                                      (bl_sb, bl_d, "bl")):
                    t = consts.tile([P, 1], FP32, name=f"{nm}{ct}")
                    nc.sync.dma_start(out=t, in_=dram[ct * P:(ct + 1) * P, :])
                    lst.append(t)
            bv_sb = consts.tile([1, C], BF16, name="bv")
            nc.sync.dma_start(out=bv_sb, in_=bv_d)
            ones_col = consts.tile([P, 1], BF16, name="ones_col")
            ones_row = consts.tile([1, P], BF16, name="ones_row")
            nc.vector.memset(ones_col, 1.0)
            nc.vector.memset(ones_row, 1.0)

            # ---- big data tiles
            xb_sb = [data.tile([P, L], BF16, name=f"xb{ct}") for ct in range(2)]
            xk_sb = [data.tile([P, CHUNK], BF16, name=f"xk{ct}") for ct in range(2)]
            xres_sb = [data.tile([P, CHUNK], FP32, name=f"xres{ct}") for ct in range(2)]
            for ct in range(2):
                nc.sync.dma_start(out=xb_sb[ct], in_=xb_d[ct * P:(ct + 1) * P, :])
                nc.sync.dma_start(out=xk_sb[ct], in_=xk_d[ct * P:(ct + 1) * P, :])
                nc.sync.dma_start(out=xres_sb[ct], in_=xres_d[ct * P:(ct + 1) * P, :])

            qp_sb = [data.tile([P, L], BF16, name=f"qp{hg}") for hg in range(2)]
            kk_sb = [data.tile([P, CHUNK], BF16, name=f"kk{hg}") for hg in range(2)]
            vt_sb = [data.tile([P, C], BF16, name=f"vt{j}") for j in range(JT)]

            # ---- projections
            with tc.tile_pool(name="pproj", bufs=2, space="PSUM") as pj:
                # K chunk [256, 1024]: needed first by the attention loop
                for ot in range(2):
                    for t2 in range(IH):
                        ps = pj.tile([P, 512], FP32, name="psk")
                        for ct in range(2):
                            nc.tensor.matmul(
                                out=ps,
                                lhsT=wk_sb[ct][:, ot * P:(ot + 1) * P],
                                rhs=xk_sb[ct][:, t2 * 512:(t2 + 1) * 512],
                                start=(ct == 0), stop=(ct == 1))
                        nc.vector.tensor_scalar(
                            out=kk_sb[ot][:, t2 * 512:(t2 + 1) * 512],
                            in0=ps, scalar1=bk_sb[ot], scalar2=None, op0=ADD)
                # Q' full [256, 4096]
                for ot in range(2):
                    for t2 in range(TS):
                        ps = pj.tile([P, 512], FP32, name="psq")
                        for ct in range(2):
                            nc.tensor.matmul(
                                out=ps,
                                lhsT=wq_sb[ct][:, ot * P:(ot + 1) * P],
                                rhs=xb_sb[ct][:, t2 * 512:(t2 + 1) * 512],
                                start=(ct == 0), stop=(ct == 1))
                        nc.vector.tensor_scalar(
                            out=qp_sb[ot][:, t2 * 512:(t2 + 1) * 512],
                            in0=ps, scalar1=bq_sb[ot], scalar2=None, op0=ADD)
                # V^T [4096, 256] with ones-row bias fold
                for j in range(JT):
                    ps = pj.tile([P, C], FP32, name="psv")
                    for ct in range(2):
                        nc.tensor.matmul(
                            out=ps,
                            lhsT=xb_sb[ct][:, j * P:(j + 1) * P],
                            rhs=wv_sb[ct],
                            start=(ct == 0), stop=False)
                    nc.tensor.matmul(out=ps, lhsT=ones_row, rhs=bv_sb,
                                     start=False, stop=True)
                    nc.vector.tensor_copy(vt_sb[j], ps)

            # ---- attention
            with tc.tile_pool(name="ring", bufs=3, space="PSUM") as ringp, \
                 tc.tile_pool(name="accp", bufs=1, space="PSUM") as accp:
                for ihalf in range(IH):
                    attn_sb = []
                    for hg in range(2):
                        acc_av = accp.tile([P, 512], FP32, name="acc_av")
                        acc_dn = accp.tile([P, 512], FP32, name="acc_dn")
                        for j in range(JT):
                            rA = ringp.tile([P, 1024], FP32, name="ring")
                            rB = ringp.tile([P, 1024], FP32, name="ring")
                            for hh in range(4):
                                rt = rA if hh < 2 else rB
                                col = (hh % 2) * 512
                                nc.tensor.matmul(
                                    out=rt[:, col:col + 512],
                                    lhsT=qp_sb[hg][32 * hh:32 * hh + 32,
                                                   j * P:(j + 1) * P],
                                    rhs=kk_sb[hg][32 * hh:32 * hh + 32,
                                                  ihalf * 512:(ihalf + 1) * 512],
                                    start=True, stop=True,
                                    tile_position=(32 * hh, 0))
                            ptA = ptp.tile([P, 1024], BF16, name="pt")
                            ptB = ptp.tile([P, 1024], BF16, name="pt")
                            nc.scalar.activation(out=ptA, in_=rA, func=EXP)
                            nc.scalar.activation(out=ptB, in_=rB, func=EXP)
                            for hh in range(4):
                                pt = ptA if hh < 2 else ptB
                                col = (hh % 2) * 512
                                h = hg * 4 + hh
                                nc.tensor.matmul(
                                    out=acc_av[32 * hh:32 * hh + 32, :],
                                    lhsT=vt_sb[j][:, 32 * h:32 * h + 32],
                                    rhs=pt[:, col:col + 512],
                                    start=(j == 0), stop=(j == JT - 1),
                                    tile_position=(0, 32 * hh))
                            for hh in range(4):
                                pt = ptA if hh < 2 else ptB
                                col = (hh % 2) * 512
                                nc.tensor.matmul(
                                    out=acc_dn[32 * hh:32 * hh + 1, :],
                                    lhsT=ones_col,
                                    rhs=pt[:, col:col + 512],
                                    start=(j == 0), stop=(j == JT - 1),
                                    tile_position=(0, 32 * hh))
                        # normalize: attn = AV / denom
                        db = ep.tile([P, 512], FP32, name="db")
                        for hh in range(4):
                            nc.vector.stream_shuffle(
                                out=db[32 * hh:32 * hh + 32, :],
                                in_=acc_dn[32 * hh:32 * hh + 32, :],
                                mask=[0] * 32)
                        rcp = ep.tile([P, 512], FP32, name="rcp")
                        nc.vector.reciprocal(rcp, db)
                        attn = attnp.tile([P, 512], BF16, name=f"attn{hg}")
                        nc.vector.tensor_mul(attn, acc_av, rcp)
                        attn_sb.append(attn)
                    # output projection + bias + residual for this i-half
                    for ot in range(2):
                        pso = ringp.tile([P, 512], FP32, name="ring")
                        for ct in range(2):
                            nc.tensor.matmul(
                                out=pso,
                                lhsT=wl_sb[ct][:, ot * P:(ot + 1) * P],
                                rhs=attn_sb[ct],
                                start=(ct == 0), stop=(ct == 1))
                        t1 = ep.tile([P, 512], FP32, name="t1")
                        nc.vector.tensor_scalar(
                            out=t1, in0=pso, scalar1=bl_sb[ot], scalar2=None,
                            op0=ADD)
                        yt = ep.tile([P, 512], FP32, name="yt")
                        nc.vector.tensor_add(
                            yt, t1,
                            xres_sb[ot][:, ihalf * 512:(ihalf + 1) * 512])
                        nc.sync.dma_start(
                            out=y_d[ot * P:(ot + 1) * P,
                                    ihalf * 512:(ihalf + 1) * 512],
                            in_=yt)

    nc.compile()
    return nc


def _get_nc():
    if "nc" not in _cache:
        _cache["nc"] = _build()
    return _cache["nc"]


def kernel(x=None, wq=None, bq=None, wk=None, bk=None, wv=None, bv=None,
           wl=None, bl=None, gamma=None, num_heads=None, **_unused):
    from concourse import bass_utils

    x = np.asarray(x, dtype=np.float32)
    assert x.shape == (N, C, H, W), f"unexpected x shape {x.shape}"
    assert int(np.asarray(num_heads)) == NH

    scale = float(CPH) ** -0.5
    g = float(np.asarray(gamma).reshape(-1)[0])

    wqsT = np.ascontiguousarray((np.asarray(wq, np.float32) * scale).T).astype(bf16)
    wkT = np.ascontiguousarray(np.asarray(wk, np.float32).T).astype(bf16)
    wvT = np.ascontiguousarray(np.asarray(wv, np.float32).T).astype(bf16)
    wlgT = np.ascontiguousarray((np.asarray(wl, np.float32) * g).T).astype(bf16)
    bq2 = (np.asarray(bq, np.float32) * scale).reshape(C, 1)
    bk2 = np.asarray(bk, np.float32).reshape(C, 1)
    bl2 = (np.asarray(bl, np.float32) * g).reshape(C, 1)
    bv2 = np.asarray(bv, np.float32).astype(bf16).reshape(1, C)

    xf = x.reshape(N, C, L)
    xbs = [xf[n].astype(bf16) for n in range(N)]

    in_maps = []
    for c in range(NCORES):
        n, ch = c // 4, c % 4
        isl = slice(ch * CHUNK, (ch + 1) * CHUNK)
        in_maps.append({
            "xb": xbs[n],
            "xk": np.ascontiguousarray(xbs[n][:, isl]),
            "xres": np.ascontiguousarray(xf[n][:, isl]),
            "wqt": wqsT, "wkt": wkT, "wvt": wvT, "wlt": wlgT,
            "bq2": bq2, "bk2": bk2, "bl2": bl2, "bv2": bv2,
        })

    nc = _get_nc()
    res = bass_utils.run_bass_kernel_spmd(nc, in_maps,
                                          core_ids=list(range(NCORES)))

    out = np.empty((N, C, L), np.float32)
    for c in range(NCORES):
        n, ch = c // 4, c % 4
        out[n][:, ch * CHUNK:(ch + 1) * CHUNK] = res.results[c]["y"]
    return out.reshape(N, C, H, W)


if __name__ == "__main__":
    import reference
    inputs = reference.setup_inputs()
    expected = np.asarray(reference.reference(**inputs))
    got = kernel(**{k: np.asarray(v) if hasattr(v, "shape") else v
                    for k, v in inputs.items()})
    rel = np.linalg.norm(got - expected) / np.linalg.norm(expected)
    print("rel err:", rel)


# revision 1
# speedup vs baseline: 1.0508x; 1.0508x over previous
"""Multi-head self-attention (SAGAN-style, 1x1-conv projections) on 8 Trainium2 cores.

Problem: x [2, 256, 64, 64], 8 heads, cph=32, L=4096 tokens per batch element.
  q/k/v = 1x1 conv projections of x; att = softmax_j(k_i . q_j); out_i = sum_j att_ij v_j;
  y = gamma * (Wl @ out + bl) + x

Sharding: output-token split — core c owns (n = c//4, token chunk c%4 of 1024).
Each core needs: full Q and V for its n, K only for its chunk. No collectives.

Per-core kernel layout (all matmuls bf16 with fp32 PSUM accumulation):
  S^T[j, i] = sum_c q'[c,j] k[c,i]   (q' pre-scaled by cph^-0.5; softmax over j)
    -> 4 heads row-packed in the 128x128 PE array (K=32 each, tile_position=(32h,0))
  P^T = exp(S^T)  on ACT, PSUM -> SBUF bf16 (scores bounded ~0.8, no max-subtraction
    needed). This is the bottleneck: 33.5M exps/core at 1 elem/cycle/lane.
  AV[d, i] = sum_j v^T[j,d] P^T[j,i]  -> 4 heads col-packed (M=32, tile_position=(0,32h))
  denom[i] = sum_j P^T[j,i]           -> ones-vector matmuls, M=1, col-packed
  attn = AV * (1/denom broadcast via stream_shuffle)
  y = Wl_gamma @ attn + bl_gamma + x_residual   (gamma folded into Wl/bl host-side)
"""
import numpy as np
import ml_dtypes

bf16 = ml_dtypes.bfloat16

N, C, H, W = 2, 256, 64, 64
L = H * W            # 4096
NH = 8               # heads
CPH = C // NH        # 32
NCORES = 8
CHUNK = 1024         # output tokens per core (L / 4)
P = 128

_cache = {}


def _build():
    import concourse.mybir as mybir
    import concourse.tile as tile
    from concourse import bacc

    FP32 = mybir.dt.float32
    BF16 = mybir.dt.bfloat16
    EXP = mybir.ActivationFunctionType.Exp
    ADD = mybir.AluOpType.add

    nc = bacc.Bacc("TRN2", target_bir_lowering=False, debug=False,
                   num_devices=NCORES)

    # ---- DRAM I/O (per-core shapes; data differs per core, program is SPMD)
    xb_d = nc.dram_tensor("xb", [C, L], BF16, kind="ExternalInput").ap()
    xk_d = nc.dram_tensor("xk", [C, CHUNK], BF16, kind="ExternalInput").ap()
    xres_d = nc.dram_tensor("xres", [C, CHUNK], FP32, kind="ExternalInput").ap()
    wq_d = nc.dram_tensor("wqt", [C, C], BF16, kind="ExternalInput").ap()
    wk_d = nc.dram_tensor("wkt", [C, C], BF16, kind="ExternalInput").ap()
    wv_d = nc.dram_tensor("wvt", [C, C], BF16, kind="ExternalInput").ap()
    wl_d = nc.dram_tensor("wlt", [C, C], BF16, kind="ExternalInput").ap()
    bq_d = nc.dram_tensor("bq2", [C, 1], FP32, kind="ExternalInput").ap()
    bk_d = nc.dram_tensor("bk2", [C, 1], FP32, kind="ExternalInput").ap()
    bl_d = nc.dram_tensor("bl2", [C, 1], FP32, kind="ExternalInput").ap()
    bv_d = nc.dram_tensor("bv2", [1, C], BF16, kind="ExternalInput").ap()
    y_d = nc.dram_tensor("y", [C, CHUNK], FP32, kind="ExternalOutput").ap()

    JT = L // P          # 32 j-tiles
    TS = L // 512        # 8 token slices for Q projection
    IH = CHUNK // 512    # 2 i-halves

    with tile.TileContext(nc) as tc:
        with tc.tile_pool(name="consts", bufs=1) as consts, \
             tc.tile_pool(name="data", bufs=1) as data, \
             tc.tile_pool(name="ptp", bufs=4) as ptp, \
             tc.tile_pool(name="ep", bufs=2) as ep, \
             tc.tile_pool(name="attnp", bufs=2) as attnp:

            # ---- constants / weights
            wq_sb, wk_sb, wv_sb, wl_sb = [], [], [], []
            bq_sb, bk_sb, bl_sb = [], [], []
            for ct in range(2):
                for lst, dram, nm in ((wq_sb, wq_d, "wq"), (wk_sb, wk_d, "wk"),
                                      (wv_sb, wv_d, "wv"), (wl_sb, wl_d, "wl")):
                    t = consts.tile([P, C], BF16, name=f"{nm}{ct}")
                    nc.sync.dma_start(out=t, in_=dram[ct * P:(ct + 1) * P, :])
                    lst.append(t)
                for lst, dram, nm in ((bq_sb, bq_d, "bq"), (bk_sb, bk_d, "bk"),
                                      (bl_sb, bl_d, "bl")):
                    t = consts.tile([P, 1], FP32, name=f"{nm}{ct}")
                    nc.sync.dma_start(out=t, in_=dram[ct * P:(ct + 1) * P, :])
                    lst.append(t)
            bv_sb = consts.tile([1, C], BF16, name="bv")
            nc.sync.dma_start(out=bv_sb, in_=bv_d)
            ones_col = consts.tile([P, 1], BF16, name="ones_col")
            ones_row = consts.tile([1, P], BF16, name="ones_row")
            nc.vector.memset(ones_col, 1.0)
            nc.vector.memset(ones_row, 1.0)

            # ---- big data tiles
            xb_sb = [data.tile([P, L], BF16, name=f"xb{ct}") for ct in range(2)]
            xk_sb = [data.tile([P, CHUNK], BF16, name=f"xk{ct}") for ct in range(2)]
            xres_sb = [data.tile([P, CHUNK], FP32, name=f"xres{ct}") for ct in range(2)]
            for ct in range(2):
                nc.sync.dma_start(out=xb_sb[ct], in_=xb_d[ct * P:(ct + 1) * P, :])
                nc.sync.dma_start(out=xk_sb[ct], in_=xk_d[ct * P:(ct + 1) * P, :])
                nc.sync.dma_start(out=xres_sb[ct], in_=xres_d[ct * P:(ct + 1) * P, :])

            qp_sb = [data.tile([P, L], BF16, name=f"qp{hg}") for hg in range(2)]
            kk_sb = [data.tile([P, CHUNK], BF16, name=f"kk{hg}") for hg in range(2)]
            vt_sb = [data.tile([P, C], BF16, name=f"vt{j}") for j in range(JT)]

            # ---- projections
            with tc.tile_pool(name="pproj", bufs=2, space="PSUM") as pj:
                # K chunk [256, 1024]: needed first by the attention loop
                for ot in range(2):
                    for t2 in range(IH):
                        ps = pj.tile([P, 512], FP32, name="psk")
                        for ct in range(2):
                            nc.tensor.matmul(
                                out=ps,
                                lhsT=wk_sb[ct][:, ot * P:(ot + 1) * P],
                                rhs=xk_sb[ct][:, t2 * 512:(t2 + 1) * 512],
                                start=(ct == 0), stop=(ct == 1))
                        nc.vector.tensor_scalar(
                            out=kk_sb[ot][:, t2 * 512:(t2 + 1) * 512],
                            in0=ps, scalar1=bk_sb[ot], scalar2=None, op0=ADD)
                # Q' full [256, 4096]
                for ot in range(2):
                    for t2 in range(TS):
                        ps = pj.tile([P, 512], FP32, name="psq")
                        for ct in range(2):
                            nc.tensor.matmul(
                                out=ps,
                                lhsT=wq_sb[ct][:, ot * P:(ot + 1) * P],
                                rhs=xb_sb[ct][:, t2 * 512:(t2 + 1) * 512],
                                start=(ct == 0), stop=(ct == 1))
                        nc.vector.tensor_scalar(
                            out=qp_sb[ot][:, t2 * 512:(t2 + 1) * 512],
                            in0=ps, scalar1=bq_sb[ot], scalar2=None, op0=ADD)
                # V^T [4096, 256] with ones-row bias fold
                for j in range(JT):
                    ps = pj.tile([P, C], FP32, name="psv")
                    for ct in range(2):
                        nc.tensor.matmul(
                            out=ps,
                            lhsT=xb_sb[ct][:, j * P:(j + 1) * P],
                            rhs=wv_sb[ct],
                            start=(ct == 0), stop=False)
                    nc.tensor.matmul(out=ps, lhsT=ones_row, rhs=bv_sb,
                                     start=False, stop=True)
                    nc.vector.tensor_copy(vt_sb[j], ps)

            # ---- attention
            with tc.tile_pool(name="ring", bufs=3, space="PSUM") as ringp, \
                 tc.tile_pool(name="accp", bufs=1, space="PSUM") as accp:
                for ihalf in range(IH):
                    attn_sb = []
                    for hg in range(2):
                        acc_av = accp.tile([P, 512], FP32, name="acc_av")
                        acc_dn = accp.tile([P, 512], FP32, name="acc_dn")
                        for j in range(JT):
                            rA = ringp.tile([P, 1024], FP32, name="ring")
                            rB = ringp.tile([P, 1024], FP32, name="ring")
                            for hh in range(4):
                                rt = rA if hh < 2 else rB
                                col = (hh % 2) * 512
                                nc.tensor.matmul(
                                    out=rt[:, col:col + 512],
                                    lhsT=qp_sb[hg][32 * hh:32 * hh + 32,
                                                   j * P:(j + 1) * P],
                                    rhs=kk_sb[hg][32 * hh:32 * hh + 32,
                                                  ihalf * 512:(ihalf + 1) * 512],
                                    start=True, stop=True,
                                    tile_position=(32 * hh, 0))
                            ptA = ptp.tile([P, 1024], BF16, name="pt")
                            ptB = ptp.tile([P, 1024], BF16, name="pt")
                            nc.scalar.activation(out=ptA, in_=rA, func=EXP)
                            nc.scalar.activation(out=ptB, in_=rB, func=EXP)
                            for hh in range(4):
                                pt = ptA if hh < 2 else ptB
                                col = (hh % 2) * 512
                                h = hg * 4 + hh
                                nc.tensor.matmul(
                                    out=acc_av[32 * hh:32 * hh + 32, :],
                                    lhsT=vt_sb[j][:, 32 * h:32 * h + 32],
                                    rhs=pt[:, col:col + 512],
                                    start=(j == 0), stop=(j == JT - 1),
                                    tile_position=(0, 32 * hh))
                            for hh in range(4):
                                pt = ptA if hh < 2 else ptB
                                col = (hh % 2) * 512
                                nc.tensor.matmul(
                                    out=acc_dn[32 * hh:32 * hh + 1, :],
                                    lhsT=ones_col,
                                    rhs=pt[:, col:col + 512],
                                    start=(j == 0), stop=(j == JT - 1),
                                    tile_position=(0, 32 * hh))
                        # normalize: attn = AV / denom
                        db = ep.tile([P, 512], FP32, name="db")
                        for hh in range(4):
                            nc.vector.stream_shuffle(
                                out=db[32 * hh:32 * hh + 32, :],
                                in_=acc_dn[32 * hh:32 * hh + 32, :],
                                mask=[0] * 32)
                        rcp = ep.tile([P, 512], FP32, name="rcp")
                        nc.vector.reciprocal(rcp, db)
                        attn = attnp.tile([P, 512], BF16, name=f"attn{hg}")
                        nc.vector.tensor_mul(attn, acc_av, rcp)
                        attn_sb.append(attn)
                    # output projection + bias + residual for this i-half
                    for ot in range(2):
                        pso = ringp.tile([P, 512], FP32, name="ring")
                        for ct in range(2):
                            nc.tensor.matmul(
                                out=pso,
                                lhsT=wl_sb[ct][:, ot * P:(ot + 1) * P],
                                rhs=attn_sb[ct],
                                start=(ct == 0), stop=(ct == 1))
                        t1 = ep.tile([P, 512], FP32, name="t1")
                        nc.vector.tensor_scalar(
                            out=t1, in0=pso, scalar1=bl_sb[ot], scalar2=None,
                            op0=ADD)
                        yt = ep.tile([P, 512], FP32, name="yt")
                        nc.vector.tensor_add(
                            yt, t1,
                            xres_sb[ot][:, ihalf * 512:(ihalf + 1) * 512])
                        nc.sync.dma_start(
                            out=y_d[ot * P:(ot + 1) * P,
                                    ihalf * 512:(ihalf + 1) * 512],
                            in_=yt)

    nc.compile()
    return nc


def _get_nc():
    if "nc" not in _cache:
        _cache["nc"] = _build()
    return _cache["nc"]


def kernel(x=None, wq=None, bq=None, wk=None, bk=None, wv=None, bv=None,
           wl=None, bl=None, gamma=None, num_heads=None, **_unused):
    from concourse import bass_utils

    x = np.asarray(x, dtype=np.float32)
    assert x.shape == (N, C, H, W), f"unexpected x shape {x.shape}"
    assert int(np.asarray(num_heads)) == NH

    scale = float(CPH) ** -0.5
    g = float(np.asarray(gamma).reshape(-1)[0])

    wqsT = np.ascontiguousarray((np.asarray(wq, np.float32) * scale).T).astype(bf16)
    wkT = np.ascontiguousarray(np.asarray(wk, np.float32).T).astype(bf16)
    wvT = np.ascontiguousarray(np.asarray(wv, np.float32).T).astype(bf16)
    wlgT = np.ascontiguousarray((np.asarray(wl, np.float32) * g).T).astype(bf16)
    bq2 = (np.asarray(bq, np.float32) * scale).reshape(C, 1)
    bk2 = np.asarray(bk, np.float32).reshape(C, 1)
    bl2 = (np.asarray(bl, np.float32) * g).reshape(C, 1)
    bv2 = np.asarray(bv, np.float32).astype(bf16).reshape(1, C)

    xf = x.reshape(N, C, L)
    xbs = [xf[n].astype(bf16) for n in range(N)]

    in_maps = []
    for c in range(NCORES):
        n, ch = c // 4, c % 4
        isl = slice(ch * CHUNK, (ch + 1) * CHUNK)
        in_maps.append({
            "xb": xbs[n],
            "xk": np.ascontiguousarray(xbs[n][:, isl]),
            "xres": np.ascontiguousarray(xf[n][:, isl]),
            "wqt": wqsT, "wkt": wkT, "wvt": wvT, "wlt": wlgT,
            "bq2": bq2, "bk2": bk2, "bl2": bl2, "bv2": bv2,
        })

    nc = _get_nc()
    res = bass_utils.run_bass_kernel_spmd(nc, in_maps,
                                          core_ids=list(range(NCORES)))

    out = np.empty((N, C, L), np.float32)
    for c in range(NCORES):
        n, ch = c // 4, c % 4
        out[n][:, ch * CHUNK:(ch + 1) * CHUNK] = res.results[c]["y"]
    return out.reshape(N, C, H, W)


if __name__ == "__main__":
    import reference
    inputs = reference.setup_inputs()
    expected = np.asarray(reference.reference(**inputs))
    got = kernel(**{k: np.asarray(v) if hasattr(v, "shape") else v
                    for k, v in inputs.items()})
    rel = np.linalg.norm(got - expected) / np.linalg.norm(expected)
    print("rel err:", rel)
